# revision 50
# baseline (speedup 1.0000x reference)
"""Trainium2 Bass kernel for a pre-norm transformer decoder layer.

Model: x = x + Attn(LN1(x));  x = x + FFN(LN2(x))
Shapes: x [2, 2048, 1024], H=16 heads, DK=64, FF=4096, f32 I/O.

Sharding over 8 NeuronCores, no collectives:
  core c -> batch entry b = c//4, query rows q0 = (c%4)*512 .. +512.
  Each core computes K/V for its full batch entry (work duplicated 4x
  within the batch group -- cheaper than an on-chip all-gather here),
  and Q/attention/FFN only for its own 512 tokens. The host slices
  inputs per core and concatenates the 8 [512,1024] outputs.

Compute: fp8(x32-scaled weights) matmuls with DoubleRow where the
contraction is >=256 (QKV, O-proj, FFN); f32 PSUM accumulation; LN
stats, softmax and residuals in f32. Attention uses S^T-layout scores
(keys on partitions) so softmax-exp output E^T feeds the PV matmul as
the STATIONARY operand (out = E^T.T-partitioned O token-major, moving
V|1 with N=65) -- 2x fewer PE cycles than moving-E. The V bias is
folded into the O-projection bias host-side (softmax rows sum to 1).
All remaining bias additions are folded into the PE as rank-1 outer
products or per-partition drain scalars.
"""

import numpy as np

import concourse.bass as bass
import concourse.mybir as mybir
import concourse.tile as tile
from concourse.masks import make_identity
from concourse.vector_clock import ScopedClock, VectorClock

F32 = mybir.dt.float32
BF16 = mybir.dt.bfloat16
F8 = mybir.dt.float8e4
AF = mybir.ActivationFunctionType
OP = mybir.AluOpType
DR = mybir.MatmulPerfMode.DoubleRow
P = 128
WS = 32.0  # fp8 weight scale
IWS = float(1.0 / WS)


def _bcast(v, p):
    """[D] AP -> [p, D] AP replicated over partitions (step-0 partition dim)."""
    return bass.AP(tensor=v.tensor, offset=v.offset, ap=[[0, p], *v.ap])


class _TC(tile.TileContext):
    """TileContext whose tail drain splits its semaphore waits across
    single-wait NOPs -- this walrus build rejects several sync waits on
    one CTRL instruction ("Too many sync wait commands")."""

    def _drain_and_barrier(self, tick_clock, wait_clock):
        gc = tick_clock.global_clock
        n = len(gc)
        for i in range(n):
            if gc[i] <= 0:
                continue
            sub = [0] * n
            sub[i] = gc[i]
            nop = self.nc.sync.nop(nofuse=True)
            wait_clock.add_sem_waits(nop.ins, ScopedClock({None: VectorClock(sub)}))
        self.nc.sync.drain()
        self.nc.all_engine_barrier()
        popped = self.nc._tile_sem_poison_stack.pop()
        assert popped is self._sem_poison
        self.nc.clear_and_free_semaphores(list(self.sems.allocated().values()))
        self.nc.all_engine_barrier()


def build_program(S=2048, D=1024, H=16, DK=64, FF=4096, Q=512, EPS=1e-6, repeat=1, phases="ABCD", timing_mode=False):
    nd = D // P        # contraction chunks over D
    ns = S // P        # token tiles (full sequence)
    nq = Q // P        # token tiles (query slice)
    nf = FF // P       # chunks over FF
    DK1 = DK + 1       # head dim + denominator column
    HPG = P // DK      # heads per 128-partition group (2)
    n_sb = max(S // 512, 1)   # 512-wide column blocks over S
    SBW = S // n_sb
    n_dh = max(D // 512, 1)   # 512-wide column blocks over D
    DHW = D // n_dh
    npr = nd // 2             # DoubleRow k-pair count over D
    assert H * DK == D and Q <= 512

    nc = bass.Bass("TRN2")

    if timing_mode:
        # inputs as internal DRAM (no host transfer) -- timing runs only
        def declare_in(name, shape, dtype):
            return nc.dram_tensor(name, shape, dtype)
    else:
        def declare_in(name, shape, dtype):
            return nc.declare_dram_parameter(name, shape, dtype, isOutput=False)

    xb = declare_in("xb", [S, D], BF16)
    xq = declare_in("xq", [Q, D], F32)
    # weights arrive pre-folded (LN affines absorbed, bv absorbed into bo)
    # and pre-cast to fp8 x32
    Wq = declare_in("Wq", [D, D], F8)
    bq = declare_in("bq", [D], F32)
    Wk = declare_in("Wk", [D, D], F8)
    bk = declare_in("bk", [D], F32)
    Wv = declare_in("Wv", [D, D], F8)
    Wo = declare_in("Wo", [D, D], F8)
    bo = declare_in("bo", [D], BF16)    # pre-scaled x32, includes bv @ Wo
    W1 = declare_in("W1", [D, FF], F8)
    b1 = declare_in("b1", [FF], F32)
    W2 = declare_in("W2", [FF, D], BF16)
    b2 = declare_in("b2", [D], BF16)
    out = nc.declare_dram_parameter("out", [Q, D], F32, isOutput=True)

    with _TC(nc) as tc:
      with tc.tile_pool(name="const", bufs=1) as cst:
        ident = cst.tile([P, P], BF16, tag="ident")
        make_identity(nc, ident)
        eps_t = cst.tile([P, 1], F32, tag="eps")
        nc.vector.memset(eps_t, EPS)
        ones_row = cst.tile([1, 512], BF16, tag="ones")
        nc.vector.memset(ones_row, 1.0)

        # per-partition layout of the FFN bias: ff = c*P + p -> [p, c]
        b1_pm = cst.tile([P, nf], F32, tag="b1pm")
        nc.sync.dma_start(out=b1_pm, in_=b1[:].rearrange("(c p) -> p c", p=P))

        # per-partition bias columns (for feature-major K/Q drains)
        bq_pm = cst.tile([P, nd], F32, tag="bqpm")
        nc.sync.dma_start(out=bq_pm, in_=bq[:].rearrange("(c p) -> p c", p=P))
        bk_pm = cst.tile([P, nd], F32, tag="bkpm")
        nc.sync.dma_start(out=bk_pm, in_=bk[:].rearrange("(c p) -> p c", p=P))
        bo_row = cst.tile([1, D], BF16, tag="bor")
        nc.sync.dma_start(out=bo_row, in_=_bcast(bo[:], 1))
        b2_row = cst.tile([1, D], BF16, tag="b2r")
        nc.sync.dma_start(out=b2_row, in_=_bcast(b2[:], 1))

        # LN1 stats pass: DMA + bn_stats only (no PSUM), batched sqrt/recip.
        # Emitted inside the PREVIOUS rep's D section so the DVE work hides
        # in D's slack instead of pacing the next rep's prelude (the engine
        # queues are in-order, so emission position IS schedule position).
        def emit_stats_pass(ridx):
            sel = ridx % 2
            mvs2 = cst.tile([P, ns, 2], F32, tag=f"mvs{sel}", name=f"mvs{sel}")
            stdt = cst.tile([P, ns], F32, tag=f"std{sel}", name=f"std{sel}")
            rstds = cst.tile([P, ns], F32, tag=f"rsd{sel}", name=f"rsd{sel}")
            nmrs = cst.tile([P, ns], F32, tag=f"nmr{sel}", name=f"nmr{sel}")
            for st in range(ns):
                x_t = cst.tile([P, D], BF16, tag="xpre", bufs=2, name="x_pre")
                nc.sync.dma_start(out=x_t, in_=xb[st * P:(st + 1) * P, :])
                xr = x_t.rearrange("p (n f) -> p n f", n=2)
                stats = cst.tile([P, 2, 6], F32, tag="st6", bufs=4, name="stats6")
                for su in range(2):
                    nc.vector.bn_stats(out=stats[:, su, :], in_=xr[:, su, :])
                nc.vector.bn_aggr(out=mvs2[:, st, :], in_=stats)
            nc.scalar.activation(out=stdt, in_=mvs2[:, :, 1], func=AF.Sqrt, bias=eps_t)
            nc.vector.reciprocal(out=rstds, in_=stdt)
            nc.vector.tensor_tensor(out=nmrs, in0=mvs2[:, :, 0], in1=rstds, op=OP.mult)
            nc.vector.tensor_scalar_mul(out=nmrs, in0=nmrs, scalar1=-1.0)
            return mvs2, rstds, nmrs

        _lnstats = {}
        for _rep in range(repeat):
            with tc.tile_pool(name="bc", bufs=1) as bcp:      # O_tok + FFN weights
              # FFN weight tiles live in bcp (spans A/B and C/D) so the
              # first-half loads can land during A/B in fresh space --
              # aliasing stream-live tiles would stall the DMAs.
              nfh = max(nf // 2, 1)        # ff chunks per half

              def load_w1_half(half):
                  tiles = []
                  for j in range(npr):
                      wbt = bcp.tile([P, 2, FF // 2], F8, tag="w1", bufs=npr + 2,
                                     name=f"w1b{half}_{j}")
                      nc.sync.dma_start(
                          out=wbt,
                          in_=W1[2 * j * P:(2 * j + 2) * P,
                                 half * (FF // 2):(half + 1) * (FF // 2)].rearrange(
                              "(two p) d -> p two d", two=2),
                      )
                      tiles.append(wbt)
                  return tiles

              def load_w2_half(half):
                  tiles = []
                  for fc in range(half * nfh, (half + 1) * nfh):
                      wbt = bcp.tile([P, D], BF16, tag="w2", bufs=nfh, name=f"w2b{fc}")
                      nc.sync.dma_start(out=wbt, in_=W2[fc * P:(fc + 1) * P, :])
                      tiles.append(wbt)
                  return tiles

              _wld = {}
              with tc.tile_pool(name="ab", bufs=1) as abp:    # KT/QT/Vt: phases A-B
                KT = [abp.tile([P, S], F8, tag=f"kt{i}", name=f"KT{i}") for i in range(nd)]
                QT = [abp.tile([P, Q], F8, tag=f"qt{i}", name=f"QT{i}") for i in range(nd)]
                Vt = [abp.tile([P, H, DK1], F8, tag=f"vt{i}", name=f"Vt{i}") for i in range(ns)]
                for st in range(ns):
                    nc.vector.memset(Vt[st][:, :, DK:DK1], 1.0)

                # ------------- Phases A+B: LN1+QKV+attention pipeline -------------
                # The host rotates each core's batch entry so the query block is
                # tokens 0..Q-1 (attention is permutation-invariant: no mask, no
                # positions), so Q^T projections read xn1T[:, :, 0:Q] directly.
                # V rides inside the LN loop (PE fills DVE-paced slack); K^T is
                # produced inside the per-head stream so exp starts early.
                O_tok = [bcp.tile([P, H, DK], BF16, tag=f"ot{i}", name=f"O_tok{i}") for i in range(nq)]
                if "A" in phases:
                    with (
                        tc.tile_pool(name="xn", bufs=2) as xnp,
                        tc.tile_pool(name="xt", bufs=1) as xtp,
                        tc.tile_pool(name="wbf", bufs=12) as wbfp,
                        tc.tile_pool(name="psA", bufs=2, space="PSUM") as psA,
                    ):
                        # xn1^T supertile [P, j-pair, slot, S] so one drain op
                        # covers 4 transposed chunks (fewer fixed-cost DVE ops)
                        xbT = xtp.tile([P, npr, 2, S], F8, tag="x1t", name="xn1T")
                        xn1T = [xbT[:, j, :, :] for j in range(npr)]

                        # QKV weights arrive pre-folded fp8 x32: k-pair tiles
                        def load_w(W_h, name):
                            wtiles = []
                            for j in range(npr):
                                wb = wbfp.tile([P, 2, D], F8, tag="wbf", name=f"wbf_{name}{j}")
                                nc.sync.dma_start(
                                    out=wb,
                                    in_=W_h[2 * j * P:(2 * j + 2) * P, :].rearrange(
                                        "(two p) d -> p two d", two=2),
                                )
                                wtiles.append(wb)
                            return wtiles

                        # filled inside the LN loop (after the first x-tile DMAs,
                        # so the input tiles win the DMA queue race)
                        Wv_bf, Wq_bf, Wk_bf = [], [], []

                        def qt_all():
                            # Q^T = Wq'^T @ xn1^T[:, 0:Q] (query block = tokens
                            # 0..Q-1 -- the host rotates the batch per core)
                            for cg in range(nd):
                                ps = psA.tile([P, Q], F32, tag="ps", bufs=2, name="ps_q")
                                for j in range(npr):
                                    nc.tensor.matmul(
                                        ps, Wq_bf[j][:, :, cg * P:(cg + 1) * P],
                                        xn1T[j][:, :, 0:Q],
                                        start=(j == 0), stop=(j == npr - 1), perf_mode=DR,
                                    )
                                if cg % 2:
                                    nc.scalar.activation(out=QT[cg], in_=ps, func=AF.Identity,
                                                         bias=bq_pm[:, cg:cg + 1], scale=IWS)
                                else:
                                    nc.vector.tensor_scalar(out=QT[cg], in0=ps,
                                                            scalar1=IWS, scalar2=bq_pm[:, cg:cg + 1],
                                                            op0=OP.mult, op1=OP.add)

                        def v_tile(st):
                            # V[st] = xn1[st] @ Wv' (token-major into [P,H,DK1])
                            for hh in range(n_dh):
                                ps = psA.tile([P, DHW], F32, tag="ps", bufs=2, name="ps_v")
                                for j in range(npr):
                                    nc.tensor.matmul(
                                        ps, xn1T[j][:, :, st * P:(st + 1) * P],
                                        Wv_bf[j][:, :, hh * DHW:(hh + 1) * DHW],
                                        start=(j == 0), stop=(j == npr - 1), perf_mode=DR,
                                    )
                                hpb = DHW // DK  # heads per column block
                                dst = Vt[st][:, hh * hpb:(hh + 1) * hpb, 0:DK]
                                src = ps.rearrange("p (h d) -> p h d", d=DK)
                                if (st + hh) % 2 and st < NVL:
                                    nc.scalar.activation(out=dst, in_=src, func=AF.Identity, scale=IWS)
                                else:
                                    nc.vector.tensor_scalar_mul(out=dst, in0=src, scalar1=IWS)

                        NVL = ns - 2  # V tiles computed inside the LN loop;
                                      # the last 2 slide into the head stream
                        if _rep not in _lnstats:
                            # rep 0 (or diagnostic phase subsets): stats here
                            _lnstats[_rep] = emit_stats_pass(_rep)
                        mvs2, rstds, nmrs = _lnstats[_rep]
                        with tc.tile_pool(name="psT", bufs=2, space="PSUM") as psT:
                            # LN1 + feature-major transpose (+ V), tile by tile
                            prex = _lnstats.get(("px", _rep))
                            for st in range(ns):
                                if prex is not None and st < len(prex):
                                    x_t = prex[st]   # DMA pre-issued in prior rep's D
                                else:
                                    x_t = cst.tile([P, D], BF16, tag="xt", bufs=4, name="x_t")
                                    nc.sync.dma_start(out=x_t, in_=xb[st * P:(st + 1) * P, :])
                                if st == 0:
                                    Wv_bf.extend(load_w(Wv, "v"))
                                elif st == 1:
                                    Wq_bf.extend(load_w(Wq, "q"))
                                elif st == 2:
                                    Wk_bf.extend(load_w(Wk, "k"))
                                xn1 = xnp.tile([P, D], BF16, tag="xn1", name="xn1")
                                if st % 2:
                                    # ACT-side normalize: Identity(x*rstd + (-mu*rstd))
                                    nc.scalar.activation(out=xn1, in_=x_t, func=AF.Identity,
                                                         scale=rstds[:, st:st + 1],
                                                         bias=nmrs[:, st:st + 1])
                                else:
                                    nc.vector.tensor_scalar(
                                        out=xn1, in0=x_t, scalar1=mvs2[:, st, 0:1],
                                        scalar2=rstds[:, st:st + 1],
                                        op0=OP.subtract, op1=OP.mult,
                                    )
                                # 8 transposed chunks, one batched drain per tile
                                pt = psT.tile([P, nd, P], BF16, tag="pt", bufs=2, name="pt")
                                for k in range(nd):
                                    nc.tensor.transpose(pt[:, k, :], xn1[:, k * P:(k + 1) * P], ident)
                                dst = xbT[:, :, :, st * P:(st + 1) * P]
                                src = pt.rearrange("p (j two) c -> p j two c", two=2)
                                if st % 2:
                                    nc.vector.tensor_copy(dst, src)
                                else:
                                    nc.scalar.activation(out=dst, in_=src, func=AF.Copy)
                                if st < NVL:
                                    v_tile(st)
                                if st == 3:
                                    qt_all()

                        # -------- attention stream: K^T -> scores -> exp -> PV --------
                        if "B" in phases:
                          with (
                            tc.tile_pool(name="psO", bufs=2, space="PSUM") as psO,
                            tc.tile_pool(name="sc", bufs=8) as scp,
                          ):
                            psS = None  # bound inside the head-loop scope below
                            kpp = 2 if ns % 2 == 0 else 1   # kt tiles per psum/exp group
                            LOOK = 2                        # pv lags scores by LOOK heads
                            NETB = (LOOK + 2) * (ns // kpp)  # live exp-tile ring

                            def kt_block(cg):
                                for tg in range(n_sb):
                                    ps = psA.tile([P, SBW], F32, tag="ps", bufs=2, name="ps_k")
                                    for j in range(npr):
                                        nc.tensor.matmul(
                                            ps, Wk_bf[j][:, :, cg * P:(cg + 1) * P],
                                            xn1T[j][:, :, tg * SBW:(tg + 1) * SBW],
                                            start=(j == 0), stop=(j == npr - 1), perf_mode=DR,
                                        )
                                    nc.vector.tensor_scalar(
                                        out=KT[cg][:, tg * SBW:(tg + 1) * SBW], in0=ps,
                                        scalar1=IWS, scalar2=bk_pm[:, cg:cg + 1],
                                        op0=OP.mult, op1=OP.add)

                            def scores(h):
                                cg, ro = h // HPG, (h % HPG) * DK
                                # E^T = exp(S^T / sqrt(DK)), S^T = K_h @ Q_h^T
                                e_tiles = []
                                for ktp in range(ns // kpp):
                                    ps = psS.tile([P, kpp, Q], F32, tag="pss", bufs=2, name="ps_s")
                                    for j in range(kpp):
                                        kt = ktp * kpp + j
                                        nc.tensor.matmul(
                                            ps[:, j, :], KT[cg][ro:ro + DK, kt * P:(kt + 1) * P],
                                            QT[cg][ro:ro + DK, :], start=True, stop=True,
                                        )
                                    et = abp.tile([P, kpp, Q], F8, tag="et", bufs=NETB, name=f"et{h}_{ktp}")
                                    nc.scalar.activation(out=et, in_=ps, func=AF.Exp, scale=float(1.0 / np.sqrt(DK)))
                                    e_tiles.append(et)
                                return e_tiles

                            def pv(h, e_tiles, tail=False, after_qc=None):
                                # O[q, d] = sum_k E^T[k, q] [V|1][k, d]: stationary
                                # E^T chunk (M=128 queries), moving V|1 (N=65) --
                                # the denominator lands in column DK; normalize
                                # per partition (query) at the drain.
                                for qc in range(nq):
                                    po = psO.tile([P, DK1], F32, tag="pso", bufs=2, name="ps_o")
                                    for kt in range(ns):
                                        nc.tensor.matmul(
                                            po, e_tiles[kt // kpp][:, kt % kpp, qc * P:(qc + 1) * P],
                                            Vt[kt][:, h, :],
                                            start=(kt == 0), stop=(kt == ns - 1),
                                        )
                                    rc = scp.tile([P, 1], F32, tag="rc", bufs=8, name="rcol")
                                    nc.vector.reciprocal(out=rc, in_=po[:, DK:DK1])
                                    if tail and qc % 2:
                                        # post-exp: the scalar engine is free again
                                        nc.scalar.activation(out=O_tok[qc][:, h, :],
                                                             in_=po[:, 0:DK],
                                                             func=AF.Identity, scale=rc)
                                    else:
                                        nc.vector.tensor_scalar_mul(
                                            out=O_tok[qc][:, h, :], in0=po[:, 0:DK], scalar1=rc)
                                    if after_qc is not None:
                                        after_qc(qc)

                            if "D" in phases:
                                # first-half FFN weights ride the stream span,
                                # when the DMA engines sit idle
                                _wld["w1a"] = load_w1_half(0)
                                _wld["w2a"] = load_w2_half(0)
                            if "C" in phases:
                                # O-projection weights too (phase C needs them
                                # immediately after the stream tail)
                                wo_t = []
                                for j in range(npr):
                                    wb = bcp.tile([P, 2, D], F8, tag="wob", bufs=npr,
                                                  name=f"wo_bf{j}")
                                    nc.sync.dma_start(
                                        out=wb,
                                        in_=Wo[2 * j * P:(2 * j + 2) * P, :].rearrange(
                                            "(two p) d -> p two d", two=2),
                                    )
                                    wo_t.append(wb)
                                _wld["Wo"] = wo_t

                            # pv lags scores by LOOK heads: the scalar engine
                            # streams exps flat-out from the first K block while
                            # the PE retires late-V tiles and lagged PV work
                            pending = []
                            with tc.tile_pool(name="psS", bufs=2, space="PSUM") as psS_:
                                psS = psS_
                                for h in range(H):
                                    if h % HPG == 0:
                                        kt_block(h // HPG)
                                    pending.append((h, scores(h)))
                                    if h < ns - NVL:
                                        v_tile(NVL + h)
                                    if len(pending) > LOOK:
                                        pv(*pending.pop(0))
                            # scores PSUM freed: transpose each O_tok q-chunk to
                            # O^T right as the final head's PV retires it, so
                            # phase C starts with its stationary operand ready
                            with tc.tile_pool(name="psTs", bufs=2, space="PSUM") as psTs:
                                O_T = [bcp.tile([P, 2, Q], F8, tag=f"otT{i}", name=f"O_T{i}")
                                       for i in range(npr)]
                                _wld["O_T"] = O_T

                                def tail_tp(qc):
                                    for k in range(nd):
                                        pt = psTs.tile([P, P], BF16, tag="pts", bufs=2, name="pts")
                                        nc.tensor.transpose(
                                            pt, O_tok[qc][:, HPG * k:HPG * (k + 1), :], ident)
                                        dst = O_T[k // 2][:, k % 2, qc * P:(qc + 1) * P]
                                        if k % 2:
                                            nc.scalar.activation(out=dst, in_=pt, func=AF.Copy)
                                        else:
                                            nc.vector.tensor_copy(dst, pt)

                                for idx, item in enumerate(pending):
                                    pv(*item, tail=True,
                                       after_qc=tail_tp if idx == len(pending) - 1 else None)

              # -------------- Phases C+D (x2 / xn2T live in both) --------------
              with tc.tile_pool(name="cd", bufs=1) as ccp:
                x2 = [ccp.tile([P, D], F32, tag=f"x2{i}", name=f"x2_{i}") for i in range(nq)]
                x2T_big = ccp.tile([P, npr, 2, Q], F8, tag="x2t", name="xn2T")
                xn2T = [x2T_big[:, j, :, :] for j in range(npr)]

                # -------------- Phase C: O-proj + residual + LN2 --------------
                if "C" in phases:
                    with (
                        tc.tile_pool(name="xioc", bufs=4) as xioc,
                        tc.tile_pool(name="psC", bufs=3, space="PSUM") as psC,
                        tc.tile_pool(name="psT3", bufs=2, space="PSUM") as psT3,
                        tc.tile_pool(name="statc", bufs=4) as stc,
                    ):
                        # O^T (feature-major, fp8 k-pairs) and the O-projection
                        # weights were staged during the stream
                        O_T = _wld["O_T"]
                        Wo_bf = _wld["Wo"]
                        for qt in range(nq):
                            xq_t = xioc.tile([P, D], F32, tag="xqc", name="xq_c")
                            nc.sync.dma_start(out=xq_t, in_=xq[qt * P:(qt + 1) * P, :])
                            for hh in range(n_dh):
                                ps = psC.tile([P, DHW], F32, tag="psc", bufs=3, name="ps_c")
                                nc.tensor.matmul(ps, ones_row[:, :P], bo_row[:, hh * DHW:(hh + 1) * DHW], start=True, stop=False)
                                for j in range(npr):
                                    nc.tensor.matmul(
                                        ps, O_T[j][:, :, qt * P:(qt + 1) * P],
                                        Wo_bf[j][:, :, hh * DHW:(hh + 1) * DHW],
                                        start=False, stop=(j == npr - 1), perf_mode=DR,
                                    )
                                nc.vector.scalar_tensor_tensor(
                                    out=x2[qt][:, hh * DHW:(hh + 1) * DHW], in0=ps,
                                    scalar=IWS, in1=xq_t[:, hh * DHW:(hh + 1) * DHW],
                                    op0=OP.mult, op1=OP.add,
                                )
                            # LN2 + affine (folded into W1/b1), then transpose
                            n_sub = max(D // 512, 1)
                            xr = x2[qt].rearrange("p (n f) -> p n f", n=n_sub)
                            stats = stc.tile([P, n_sub, 6], F32, tag="st2", bufs=4, name="stats2")
                            for su in range(n_sub):
                                nc.vector.bn_stats(out=stats[:, su, :], in_=xr[:, su, :])
                            mv = stc.tile([P, 2], F32, tag="mv2", bufs=4, name="mv2")
                            nc.vector.bn_aggr(out=mv, in_=stats)
                            stdt = stc.tile([P, 1], F32, tag="sd2", bufs=4, name="stdt2")
                            nc.scalar.activation(out=stdt, in_=mv[:, 1:2], func=AF.Sqrt, bias=eps_t)
                            rstd = stc.tile([P, 1], F32, tag="rs2", bufs=4, name="rstd2")
                            nc.vector.reciprocal(out=rstd, in_=stdt)
                            xn2 = stc.tile([P, D], BF16, tag="xn2", bufs=2, name="xn2")
                            if qt % 2:
                                nmr2 = stc.tile([P, 1], F32, tag="nm2", bufs=4, name="nmr2")
                                nc.vector.tensor_scalar(
                                    out=nmr2, in0=mv[:, 0:1], scalar1=-1.0, scalar2=rstd,
                                    op0=OP.mult, op1=OP.mult)
                                nc.scalar.activation(out=xn2, in_=x2[qt], func=AF.Identity,
                                                     scale=rstd, bias=nmr2)
                            else:
                                nc.vector.tensor_scalar(
                                    out=xn2, in0=x2[qt], scalar1=mv[:, 0:1], scalar2=rstd,
                                    op0=OP.subtract, op1=OP.mult,
                                )
                            pt = psT3.tile([P, nd, P], BF16, tag="pt3", bufs=2, name="pt3")
                            for k in range(nd):
                                nc.tensor.transpose(pt[:, k, :], xn2[:, k * P:(k + 1) * P], ident)
                            dst = x2T_big[:, :, :, qt * P:(qt + 1) * P]
                            src = pt.rearrange("p (j two) c -> p j two c", two=2)
                            if qt % 2:
                                nc.vector.tensor_copy(dst, src)
                            else:
                                nc.scalar.activation(out=dst, in_=src, func=AF.Copy)

                # ---------------- Phase D: FFN + residual + out ----------------
                if "D" in phases:
                    with (
                        tc.tile_pool(name="h1", bufs=1) as h1p,
                        tc.tile_pool(name="y2a", bufs=1) as y2p,
                        tc.tile_pool(name="od", bufs=2) as odp,
                        tc.tile_pool(name="psH", bufs=2, space="PSUM") as psH,
                        tc.tile_pool(name="psY", bufs=2, space="PSUM") as psY,
                    ):
                        # h1^T bf16 (the W2 matmuls stay bf16 for accuracy)
                        h1T = [h1p.tile([P, Q], BF16, tag=f"h1{i}", name=f"h1T{i}") for i in range(nf)]
                        y2a = [y2p.tile([P, D], F32, tag=f"ya{i}", name=f"y2a{i}") for i in range(nq)]

                        def h1_half(w1_tiles, half):
                            for fc in range(half * nfh, (half + 1) * nfh):
                                ps = psH.tile([P, Q], F32, tag="psh", bufs=3, name="ps_h")
                                lc = fc - half * nfh
                                for j in range(npr):
                                    nc.tensor.matmul(
                                        ps, w1_tiles[j][:, :, lc * P:(lc + 1) * P], xn2T[j],
                                        start=(j == 0), stop=(j == npr - 1), perf_mode=DR,
                                    )
                                nc.scalar.activation(
                                    out=h1T[fc], in_=ps, func=AF.Relu,
                                    bias=b1_pm[:, fc:fc + 1], scale=IWS,
                                )

                        w1a, w2a = _wld["w1a"], _wld["w2a"]
                        h1_half(w1a, 0)
                        if "A" in phases and _rep + 1 < repeat:
                            # next rep's LN1 stats ride phase D's DVE/DMA slack
                            _lnstats[_rep + 1] = emit_stats_pass(_rep + 1)
                            px = []
                            for st in range(4):
                                x_t = cst.tile([P, D], BF16, tag="xt", bufs=4, name="x_t")
                                nc.sync.dma_start(out=x_t, in_=xb[st * P:(st + 1) * P, :])
                                px.append(x_t)
                            _lnstats[("px", _rep + 1)] = px
                        w1b = load_w1_half(1)
                        w2b = load_w2_half(1)
                        # y2a = h1[:, :FF/2] @ W2[:FF/2] (drained to SBUF)
                        for qt in range(nq):
                            for hh in range(n_dh):
                                ps = psY.tile([P, DHW], F32, tag="psy", bufs=2, name="ps_ya")
                                for i, fc in enumerate(range(0, nfh)):
                                    nc.tensor.matmul(
                                        ps, h1T[fc][:, qt * P:(qt + 1) * P],
                                        w2a[i][:, hh * DHW:(hh + 1) * DHW],
                                        start=(i == 0), stop=(i == nfh - 1),
                                    )
                                dst = y2a[qt][:, hh * DHW:(hh + 1) * DHW]
                                if qt % 2:
                                    nc.scalar.activation(out=dst, in_=ps, func=AF.Copy)
                                else:
                                    nc.vector.tensor_copy(dst, ps)
                        h1_half(w1b, 1)
                        # y2a += x2 (so the final drain is one fused op)
                        for qt in range(nq):
                            nc.vector.tensor_tensor(out=y2a[qt], in0=y2a[qt], in1=x2[qt], op=OP.add)
                        for qt in range(nq):
                            o_t = odp.tile([P, D], F32, tag="od", name="o_t")
                            for hh in range(n_dh):
                                ps = psY.tile([P, DHW], F32, tag="psy", bufs=2, name="ps_yb")
                                nc.tensor.matmul(ps, ones_row[:, :P], b2_row[:, hh * DHW:(hh + 1) * DHW], start=True, stop=False)
                                for i, fc in enumerate(range(nfh, nf)):
                                    nc.tensor.matmul(
                                        ps, h1T[fc][:, qt * P:(qt + 1) * P],
                                        w2b[i][:, hh * DHW:(hh + 1) * DHW],
                                        start=False, stop=(i == nfh - 1),
                                    )
                                sl = slice(hh * DHW, (hh + 1) * DHW)
                                nc.vector.tensor_tensor(
                                    out=o_t[:, sl], in0=ps, in1=y2a[qt][:, sl], op=OP.add)
                            nc.sync.dma_start(out=out[qt * P:(qt + 1) * P, :], in_=o_t)

    return nc


_MAXW = 1  # max sync waits walrus accepts per instruction here


def _split_waits_json(raw: bytes) -> bytes:
    """Split multi-wait instructions: excess sync waits move onto
    preceding single-wait EventSemaphore instructions on the same
    engine (the engine stalls there, gating everything it issues
    afterwards -- semantically identical, codegen-legal)."""
    import json as _json

    d = _json.loads(raw)
    ctr = 0
    for f in d.get("functions", []):
        for bb in f.get("blocks", []):
            insts = bb.get("instructions", [])
            out = []
            for ins in insts:
                si = ins.get("sync_info")
                waits = si.get("on_wait") if si else None
                if waits and len(waits) > _MAXW:
                    for w in waits[:-_MAXW]:
                        ctr += 1
                        out.append({
                            "debug": ins.get("debug", 0),
                            "engine": ins["engine"],
                            "ins": [],
                            "outs": [],
                            "name": f"wsplit-{ctr}",
                            "opcode": "EventSemaphore",
                            "sync_info": {"on_update": [], "on_wait": [w]},
                        })
                    si["on_wait"] = waits[-_MAXW:]
                out.append(ins)
            bb["instructions"] = out
    return _json.dumps(d).encode()


def _patch_serialization(nc):
    orig = nc.to_json_bytes

    def patched():
        return _split_waits_json(orig())

    nc.to_json_bytes = patched
    return nc


_CACHED = {}


def _get_nc():
    if "nc" not in _CACHED:
        _CACHED["nc"] = _patch_serialization(build_program())
    return _CACHED["nc"]


def fold_weights(inputs):
    """Host-side prep: absorb the LN affines into adjacent weights/biases
    (exact f32 algebra), fold bv into bo (softmax rows sum to 1, so the
    V bias contributes bv @ Wo to every output row), then cast weights
    to x32-scaled fp8 for the TensorEngine.

      LN(x) @ W + b = z @ (g*W) + (ln_b @ W + b),  z = (x-mu)*rstd
    """
    import ml_dtypes

    f = lambda k: np.asarray(inputs[k], dtype=np.float32)
    bf = lambda a: np.ascontiguousarray(np.asarray(a, np.float32).astype(ml_dtypes.bfloat16))
    f8 = lambda a: np.ascontiguousarray(
        (np.asarray(a, np.float32) * WS).astype(ml_dtypes.float8_e4m3))
    fc = lambda a: np.ascontiguousarray(np.asarray(a, np.float32))
    g1, l1b = f("ln1_g"), f("ln1_b")
    g2, l2b = f("ln2_g"), f("ln2_b")
    out = {}
    for nm, bnm in (("Wq", "bq"), ("Wk", "bk")):
        W = f(nm)
        out[nm] = f8(g1[:, None] * W)
        out[bnm] = fc(l1b @ W + f(bnm))
    Wv = f("Wv")
    out["Wv"] = f8(g1[:, None] * Wv)
    bv_eff = (l1b @ Wv + f("bv"))
    Wo = f("Wo")
    out["Wo"] = f8(Wo)
    out["bo"] = bf((f("bo") + bv_eff @ Wo) * WS)
    W1 = f("W1")
    out["W1"] = f8(g2[:, None] * W1)
    out["b1"] = np.ascontiguousarray(l2b @ W1 + f("b1"))
    out["W2"] = bf(f("W2"))
    out["b2"] = bf(f("b2"))
    return out


def make_in_maps(inputs):
    x = np.ascontiguousarray(np.asarray(inputs["x"], dtype=np.float32))
    B, S, D = x.shape
    QW = B * S // 8
    shared = fold_weights(inputs)
    gpb = 8 // B  # cores per batch entry
    in_maps = []
    for c in range(8):
        b, g = c // gpb, c % gpb
        m = dict(shared)
        # rotate so this core's query block sits at rows 0..QW-1 (attention
        # is permutation-invariant over tokens: no mask, no positions)
        import ml_dtypes
        m["xb"] = np.ascontiguousarray(np.roll(x[b], -g * QW, axis=0).astype(ml_dtypes.bfloat16))
        m["xq"] = np.ascontiguousarray(x[b][g * QW:(g + 1) * QW])
        in_maps.append(m)
    return in_maps


def kernel(**inputs) -> np.ndarray:
    from concourse.bass_utils import run_bass_kernel_spmd

    x = np.asarray(inputs["x"])
    B, S, D = x.shape
    QW = B * S // 8
    gpb = 8 // B
    nc = _get_nc()
    res = run_bass_kernel_spmd(nc, make_in_maps(inputs), core_ids=list(range(8)))
    out = np.empty((B, S, D), dtype=np.float32)
    for c in range(8):
        b, g = c // gpb, c % gpb
        out[b, g * QW:(g + 1) * QW] = res.results[c]["out"]
    return out
